# revision 46
# baseline (speedup 1.0000x reference)
"""CLVP attention kernel for 8 Trainium2 NeuronCores.

Problem: B=2, T=2048, E=768, H=12 heads of HD=64; rotary on first 32 dims
of q, k AND v; softmax attention; output projection.

Sharding: the 24 (batch, head) pairs are split 3-heads-x-1-batch per core
(core c: batch c//4, heads 3*(c%4)..3*(c%4)+2).  Wq/Wk/Wv are split
column-wise (by head), Wo row-wise, so each core produces a partial
(T, E) output for its batch; the host sums the 4 partials per batch
(row-parallel tensor parallelism) and adds bo.

All matmul operands are bf16 (host casts x and weights; on-chip
intermediates are written back as bf16).  On TRN2 a bf16 moving operand
streams 1 column/cycle vs 2 for f32r at K=128, halving PE time for the
projection, AV and output-projection matmuls.  PSUM accumulation stays
f32, and the softmax denominator is accumulated exactly from the same
bf16 e values used in the numerator (ones-column trick), so the only
accuracy loss is bf16 rounding of inputs/intermediates (~0.5% rel).

Layout/scheduling notes:
  - HBM loads use >=2KB per-partition lines (weights host-packed to
    [128, 6*cols]); the DMA path here is packet-rate-bound, so line
    width ~= bandwidth.
  - qkv projection in natural layout (t on partitions), RoPE via
    strided-AP DVE/Pool ops, q/k transposed via PE identity matmuls.
  - v (+ones), q/k/out transposed tiles are SPLIT per t-quarter/half:
    the Tile framework tracks whole-tile deps, so splitting is what
    lets attention start before the last RoPE pass and lets the output
    projection of half 0 overlap the last head's second half.
  - softmax denominator: PSUM row 64 -> DVE copy to SBUF -> GpSimd
    partition_broadcast across 64 partitions -> DVE fast approximate
    reciprocal (base partition 0) -> DVE multiply (bf16 out).  All
    on-chip; no DMA on the critical path.
  - PSUM budget 8 banks: scores/proj share one [128,1024] ring (4) +
    out.T accumulators (4).
"""

import numpy as np
import ml_dtypes

try:
    import concourse.bass as bass
except ImportError:
    import sys
    for p in ("/opt/trn_rl_repo", "/root/.axon_site/_ro/trn_rl_repo"):
        if p not in sys.path:
            sys.path.insert(0, p)
    import concourse.bass as bass

import concourse.tile as tile
from concourse import bacc, mybir, bass_utils
from concourse.masks import make_identity

F32 = mybir.dt.float32
BF16 = mybir.dt.bfloat16
AF = mybir.ActivationFunctionType
OP = mybir.AluOpType

B, T, E, H, HD, ROT = 2, 2048, 768, 12, 64, 32
SCALE = HD ** -0.5
HPC = 3                      # heads per core
NC_CORES = 8
TCH = T // 128               # 16 t-chunks
ECH = E // 128               # 6 embedding chunks
QK = 2 * HPC * HD            # 384  (q rows + k rows per core)
VW = HPC * (HD + 1)          # 195  (v + ones column per head) per chunk


def _v(t, offset, dims):
    """AP view of tile t at element offset with free dims [[step, count], ...]."""
    return bass.AP(tensor=t.tensor, offset=t.offset + offset,
                   ap=[t.ap[0]] + [list(d) for d in dims])


def build_nc():
    nc = bacc.Bacc("TRN2", target_bir_lowering=False, debug=False)

    xT_d = nc.dram_tensor("xT", [E, T], BF16, kind="ExternalInput").ap()
    wqk_d = nc.dram_tensor("wqk", [128, ECH * QK], BF16, kind="ExternalInput").ap()
    wv_d = nc.dram_tensor("wv", [128, ECH * HPC * HD], BF16, kind="ExternalInput").ap()
    wo_d = nc.dram_tensor("wo", [HPC * HD, E], BF16, kind="ExternalInput").ap()
    fr_d = nc.dram_tensor("fr", [128, TCH * ROT], F32, kind="ExternalInput").ap()
    y_d = nc.dram_tensor("y", [T, E], BF16, kind="ExternalOutput").ap()

    with tile.TileContext(nc) as tc:
        with tc.tile_pool(name="persist", bufs=1) as P:
            # ---- persistent SBUF tiles ----
            # wqk as 2x3 chunks (split so the first matmul waits only 1 DMA)
            wqk_t = [P.tile([128, ECH // 2, QK], BF16, name=f"wqk{p}")
                     for p in range(2)]
            wv_t = P.tile([128, ECH, HPC * HD], BF16)
            wo_a = P.tile([128, E], BF16)           # heads 0,1 of woT
            wo_b = P.tile([64, E], BF16)            # head 2
            fr_t = P.tile([128, TCH * ROT], F32)
            cos_t = P.tile([128, TCH * ROT], BF16)
            sin_t = P.tile([128, TCH * ROT], BF16)
            ident = P.tile([128, 128], BF16)
            ident_f = P.tile([128, 128], F32)
            halfpi = P.tile([128, 1], F32)
            qk_nat = P.tile([128, TCH * QK], BF16)  # q|k natural, 6144
            # v+ones per t-quarter; q/k/out transposed (d-major) per T-half
            vt_q = [P.tile([128, 4 * VW], BF16, name=f"vt{q}") for q in range(4)]
            qT_a = [P.tile([128, T // 2], BF16, name=f"qTa{s}") for s in range(2)]
            qT_b = [P.tile([64, T // 2], BF16, name=f"qTb{s}") for s in range(2)]
            kT_a = [P.tile([128, T // 2], BF16, name=f"kTa{s}") for s in range(2)]
            kT_b = [P.tile([64, T // 2], BF16, name=f"kTb{s}") for s in range(2)]
            oT_a = [P.tile([128, T // 2], BF16, name=f"oTa{s}") for s in range(2)]
            oT_b = [P.tile([64, T // 2], BF16, name=f"oTb{s}") for s in range(2)]

            make_identity(nc, ident_f)
            nc.vector.tensor_copy(ident, ident_f)
            nc.gpsimd.memset(halfpi, float(np.pi / 2))
            # memset bf16 1.0 bit pattern for the ones columns
            for q in range(4):
                nc.gpsimd.memset(vt_q[q].bitcast(mybir.dt.uint16), 0x3F80)

            # ---- phases C+D: qkv projection, RoPE, transposes, pipelined
            # per t-quarter so PE/DVE/Pool overlap ----
            with tc.tile_pool(name="xt", bufs=1) as XP, \
                 tc.tile_pool(name="rope_scr", bufs=2) as RS, \
                 tc.tile_pool(name="psC", bufs=3, space="PSUM") as PSC, \
                 tc.tile_pool(name="psD", bufs=2, space="PSUM") as PSD:
                # x split per (e-chunk, T-half): whole-tile dep granularity,
                # so the first matmul only waits for its own 256KB DMA.
                xt = [[XP.tile([128, T // 2], BF16, name=f"xt{j}_{s}")
                       for s in range(2)] for j in range(ECH)]

                def rope_q(eng, base_t, blk_w, nblk, q, scr, off=None,
                           n4=4, cb=0):
                    h = 16
                    A = scr.tile([128, n4 * nblk * h], BF16, tag=f"ropeA{blk_w}{n4}")
                    Bv = scr.tile([128, n4 * nblk * h], BF16, tag=f"ropeB{blk_w}{n4}")
                    if off is None:
                        off = q * 4 * blk_w * nblk
                    off += cb * blk_w * nblk
                    Q1 = _v(base_t, off, [[blk_w * nblk, n4], [blk_w, nblk], [1, h]])
                    Q2 = _v(base_t, off + h, [[blk_w * nblk, n4], [blk_w, nblk], [1, h]])
                    Av = _v(A, 0, [[nblk * h, n4], [h, nblk], [1, h]])
                    Bvv = _v(Bv, 0, [[nblk * h, n4], [h, nblk], [1, h]])
                    fo = (q * 4 + cb) * ROT
                    C1 = _v(cos_t, fo, [[ROT, n4], [0, nblk], [1, h]])
                    C2 = _v(cos_t, fo + h, [[ROT, n4], [0, nblk], [1, h]])
                    S1 = _v(sin_t, fo, [[ROT, n4], [0, nblk], [1, h]])
                    S2 = _v(sin_t, fo + h, [[ROT, n4], [0, nblk], [1, h]])
                    eng.tensor_tensor(Av, Q2, S1, OP.mult)
                    eng.tensor_tensor(Bvv, Q1, S2, OP.mult)
                    eng.tensor_tensor(Q1, Q1, C1, OP.mult)
                    eng.tensor_tensor(Q1, Q1, Av, OP.subtract)
                    eng.tensor_tensor(Q2, Q2, C2, OP.mult)
                    eng.tensor_tensor(Q2, Q2, Bvv, OP.add)

                # loads, 2KB per-partition lines throughout; ordered so the
                # first matmul (needs x j0 half0 + wqk) can start earliest.
                nc.sync.dma_start(out=xt[0][0],
                                  in_=xT_d[0:128, 0:1024])
                nc.sync.dma_start(out=wqk_t[0], in_=wqk_d[:, 0:3 * QK])
                nc.sync.dma_start(out=xt[1][0],
                                  in_=xT_d[128:256, 0:1024])
                nc.sync.dma_start(out=wv_t, in_=wv_d)
                nc.sync.dma_start(out=wqk_t[1], in_=wqk_d[:, 3 * QK:])
                nc.sync.dma_start(out=fr_t, in_=fr_d)
                nc.scalar.activation(cos_t, fr_t, AF.Sin,
                                     bias=halfpi[:, :], scale=1.0)
                nc.scalar.activation(sin_t, fr_t, AF.Sin, scale=1.0)
                for j in range(2, ECH):
                    nc.sync.dma_start(out=xt[j][0],
                                      in_=xT_d[j * 128:(j + 1) * 128, 0:1024])
                for j in range(ECH):
                    nc.sync.dma_start(out=xt[j][1],
                                      in_=xT_d[j * 128:(j + 1) * 128, 1024:2048])
                nc.sync.dma_start(out=wo_a, in_=wo_d[0:128, :])
                nc.sync.dma_start(out=wo_b, in_=wo_d[128:192, :])

                def transp_q(ii, ceng):
                    for i in ii:
                        col = i * QK
                        s, ts_sl = i // 8, slice((i % 8) * 128, (i % 8) * 128 + 128)
                        pt = PSD.tile([128, 512], BF16, tag="tr", name="pt")
                        nc.tensor.transpose(pt[:, 0:128], qk_nat[:, col:col + 128], ident)
                        nc.tensor.transpose(pt[0:64, 128:256], qk_nat[:, col + 128:col + 192], ident)
                        nc.tensor.transpose(pt[:, 256:384], qk_nat[:, col + 192:col + 320], ident)
                        nc.tensor.transpose(pt[0:64, 384:512], qk_nat[:, col + 320:col + 384], ident)
                        ceng(qT_a[s][:, ts_sl], pt[:, 0:128])
                        ceng(qT_b[s][:, ts_sl], pt[0:64, 128:256])
                        ceng(kT_a[s][:, ts_sl], pt[:, 256:384])
                        ceng(kT_b[s][:, ts_sl], pt[0:64, 384:512])

                vt_pend = []

                def c_block(i, q):
                    ps_qk = PSC.tile([128, QK], F32, tag="qk")
                    ps_v = PSC.tile([128, HPC * HD], F32, tag="v")
                    for j in range(ECH):
                        lhs = xt[j][i // 8][:, (i % 8) * 128:(i % 8) * 128 + 128]
                        nc.tensor.matmul(ps_qk, lhs, wqk_t[j // 3][:, j % 3, :],
                                         start=(j == 0), stop=(j == ECH - 1))
                        nc.tensor.matmul(ps_v, lhs, wv_t[:, j, :],
                                         start=(j == 0), stop=(j == ECH - 1))
                    nc.vector.tensor_copy(qk_nat[:, i * QK:(i + 1) * QK], ps_qk)
                    # v columns into 65-wide head blocks (ones col untouched);
                    # deferred until after the q/k RoPE so the DVE reaches the
                    # RoPE ops sooner (v's RoPE runs on GpSimd)
                    dst = _v(vt_q[q], (i - 4 * q) * VW, [[HD + 1, HPC], [1, HD]])
                    src = _v(ps_v, 0, [[HD, HPC], [1, HD]])
                    vt_pend.append((dst, src))

                def flush_vt():
                    for dst, src in vt_pend:
                        nc.vector.tensor_copy(dst, src)
                    vt_pend.clear()

                for q in range(3):
                    for i in range(4 * q, 4 * q + 4):
                        c_block(i, q)
                    rope_q(nc.vector, qk_nat, HD, 2 * HPC, q, RS)
                    flush_vt()
                    rope_q(nc.gpsimd, vt_q[q], HD + 1, HPC, q, RS, off=0)
                    # transposes run one quarter behind so the PE never waits
                    # for the current quarter's RoPE pass; copies on ACT
                    # (idle during projection)
                    if q >= 1:
                        transp_q(range(4 * q - 4, 4 * q), nc.scalar.copy)

                # last quarter: pair-interleave copies and RoPE on the DVE so
                # the transposes have their inputs the moment the PE is free;
                # q2's transposes slot between the matmul blocks so their ACT
                # copies drain before attention's exps start.
                c_block(12, 3)
                c_block(13, 3)
                rope_q(nc.vector, qk_nat, HD, 2 * HPC, 3, RS, n4=2, cb=0)
                transp_q((8, 9), nc.scalar.copy)
                c_block(14, 3)
                c_block(15, 3)
                rope_q(nc.vector, qk_nat, HD, 2 * HPC, 3, RS, n4=2, cb=2)
                flush_vt()
                rope_q(nc.gpsimd, vt_q[3], HD + 1, HPC, 3, RS, off=0)
                transp_q((10, 11), nc.scalar.copy)
                # keep-warm fillers bridge the last RoPE wait
                for w in range(30):
                    fill = PSD.tile([128, 128], BF16, tag="tr", name="fill")
                    nc.tensor.transpose(
                        fill, qk_nat[:, (w % 8) * 128:(w % 8) * 128 + 128], ident)
                transp_q(range(12, 16), nc.vector.tensor_copy)

            # ---- phase E: attention per head; phase F (output projection)
            # reuses the scores PSUM ring so half 0 overlaps the last head's
            # second half.  PSUM budget: PSS 4 + PSO 4 = 8 banks. ----
            with tc.tile_pool(name="psS", bufs=2, space="PSUM") as PSS, \
                 tc.tile_pool(name="psO", bufs=2, space="PSUM") as PSO, \
                 tc.tile_pool(name="epool", bufs=3) as EP, \
                 tc.tile_pool(name="ysb", bufs=4) as YSB, \
                 tc.tile_pool(name="rcp", bufs=4) as RCP:
                def proj_m(half, mm, ceng):
                    # one output-projection t-chunk; ps_y comes from the
                    # scores ring (all 4 matmuls before the single strided
                    # PSUM->SBUF cast so nothing serializes the banks)
                    msl = slice(mm * 128, (mm + 1) * 128)
                    ysl = slice(half * 1024 + mm * 128,
                                half * 1024 + (mm + 1) * 128)
                    y_s = YSB.tile([128, E], BF16, tag="ys", name="y_s")
                    ps_y = PSS.tile([128, T // 2], F32, tag="ps", name="ps_y")
                    for osl, csl in ((slice(0, 384), slice(0, 384)),
                                     (slice(512, 896), slice(384, E))):
                        nc.tensor.matmul(ps_y[:, osl], oT_a[half][:, msl],
                                         wo_a[:, csl], start=True, stop=False)
                        nc.tensor.matmul(ps_y[:, osl], oT_b[half][:, msl],
                                         wo_b[:, csl], start=False, stop=True)
                    ceng(y_s, _v(ps_y, 0, [[512, 2], [1, 384]]))
                    nc.sync.dma_start(out=y_d[ysl, :], in_=y_s)

                for h in range(HPC):
                    psl = slice(0, 64) if h != 1 else slice(64, 128)

                    for half in range(2):
                        qT_h = (qT_a if h < 2 else qT_b)[half][psl, :]
                        oT_dst = (oT_a if h < 2 else oT_b)[half][psl, :]
                        ps_o = PSO.tile([HD + 1, T // 2], F32, tag="po")
                        for m in range(TCH):
                            kT_h = (kT_a if h < 2 else kT_b)[m // 8][psl, :]
                            ksl = slice((m % 8) * 128, (m % 8) * 128 + 128)
                            e_m = EP.tile([128, T // 2], BF16, tag="e")
                            ps_s = PSS.tile([128, T // 2], F32, tag="ps")
                            for n in range(2):
                                nsl = slice(n * 512, (n + 1) * 512)
                                nc.tensor.matmul(
                                    ps_s[:, nsl], kT_h[:, ksl], qT_h[:, nsl],
                                    start=True, stop=True)
                            nc.scalar.activation(e_m, ps_s, AF.Exp)
                            vh = _v(vt_q[m // 4],
                                    (m % 4) * VW + h * (HD + 1), [[1, HD + 1]])
                            for n in range(2):
                                nsl = slice(n * 512, (n + 1) * 512)
                                nc.tensor.matmul(
                                    ps_o[:, nsl], vh, e_m[:, nsl],
                                    start=(m == 0), stop=(m == TCH - 1))


                        # deferred normalization, all on-chip: copy the
                        # denominator row to SBUF, broadcast it across 64
                        # partitions on the (idle) GpSimd engine, take the
                        # fast approximate reciprocal on 64 lanes (base
                        # partition 0 — the custom DVE op breaks at base 64),
                        # multiply into bf16 out.T.
                        den_h = RCP.tile([1, T // 2], F32, tag="dn")
                        nc.vector.tensor_copy(den_h, ps_o[HD:HD + 1, :])
                        rb_h = RCP.tile([64, T // 2], F32, tag="rb")
                        rr_h = RCP.tile([64, T // 2], F32, tag="rr")
                        nc.gpsimd.partition_broadcast(rb_h, den_h, channels=64)
                        nc.vector.reciprocal_approx_fast(out=rr_h, in_=rb_h)
                        nc.vector.tensor_tensor(oT_dst, ps_o[0:HD, :],
                                                rr_h, OP.mult)

                # ---- phase F: half 0 projects right after attention ends
                # (its normalization landed half a head ago); its casts run
                # on ACT (free after the last exp), pacing the PE at ~52%
                # while the last normalization chain drains on GpSimd+DVE.
                # Half 1 then projects at full PE pace, casts alternating.
                for mm in range(8):
                    proj_m(0, mm, nc.scalar.copy)

                for mm in range(8):
                    proj_m(1, mm,
                           nc.scalar.copy if mm % 2 else nc.vector.tensor_copy)

    nc.compile()
    return nc


_NC_CACHE = None


def _get_nc():
    global _NC_CACHE
    if _NC_CACHE is None:
        _NC_CACHE = build_nc()
    return _NC_CACHE


def make_in_maps(hidden_states, rotary_pos_emb, Wq, Wk, Wv, Wo):
    bf = ml_dtypes.bfloat16
    fr = np.ascontiguousarray(
        rotary_pos_emb.reshape(TCH, 128, ROT).transpose(1, 0, 2).reshape(128, TCH * ROT),
        dtype=np.float32)
    in_maps = []
    for c in range(NC_CORES):
        b = c // (NC_CORES // B)
        g = c % (NC_CORES // B)
        rows = slice(HPC * HD * g, HPC * HD * (g + 1))
        xT = np.ascontiguousarray(hidden_states[b].T).astype(bf)
        # host-packed [128, 6*cols] so each is a single wide-line DMA
        wqk = np.concatenate([Wq[rows].T * SCALE, Wk[rows].T], axis=1).astype(bf)
        wqk = np.ascontiguousarray(
            wqk.reshape(ECH, 128, QK).transpose(1, 0, 2).reshape(128, ECH * QK))
        wv = np.ascontiguousarray(Wv[rows].T).astype(bf)
        wv = np.ascontiguousarray(
            wv.reshape(ECH, 128, HPC * HD).transpose(1, 0, 2).reshape(128, -1))
        wo = np.ascontiguousarray(Wo[:, rows].T).astype(bf)
        in_maps.append({
            "xT": xT,
            "wqk": wqk,
            "wv": wv,
            "wo": wo,
            "fr": fr,
        })
    return in_maps


def kernel(hidden_states, rotary_pos_emb, Wq, Wk, Wv, Wo, bo):
    hidden_states = np.asarray(hidden_states, dtype=np.float32)
    rotary_pos_emb = np.asarray(rotary_pos_emb, dtype=np.float32)
    Wq = np.asarray(Wq, dtype=np.float32)
    Wk = np.asarray(Wk, dtype=np.float32)
    Wv = np.asarray(Wv, dtype=np.float32)
    Wo = np.asarray(Wo, dtype=np.float32)
    bo = np.asarray(bo, dtype=np.float32)

    from concourse import bass_utils as _bu
    nc = _get_nc()
    in_maps = make_in_maps(hidden_states, rotary_pos_emb.reshape(T, ROT),
                           Wq, Wk, Wv, Wo)
    res = _bu.run_bass_kernel_spmd(
        nc, in_maps, core_ids=list(range(NC_CORES)), trace=False)

    out = np.zeros((B, T, E), dtype=np.float32)
    for c in range(NC_CORES):
        out[c // (NC_CORES // B)] += np.asarray(res.results[c]["y"], dtype=np.float32)
    out += bo
    return out
